# revision 22
# baseline (speedup 1.0000x reference)
"""Causal self-attention on 8 TRN2 NeuronCores (Bass/Tile, SPMD).

Problem: B=4, T=2048, C=1024, NH=16, HS=64.
  qkv = x @ W_attn + b_attn; causal softmax attention per head; y @ W_proj + b_proj.

Sharding: core = (batch b, class xh) with b = core//2, xh = core%2.
Each core computes qkv (Q^T only for its own queries) for its whole batch,
then attention + output projection for 1024 of its batch's queries: the two
512-token blocks {0,3} (class A) or {1,2} (class B) -- paired so causal work
is balanced across cores.

SPMD uniformity: all 8 cores run the *same* instruction stream; per-class
differences are absorbed into data (per-core 128-token-tile permutation of
x, shared position-universal diagonal masks, per-core 0/1 blend scalars).

v3 flex-slot schedule (20 attention slots per head-pack, NO dead slots; v2
had 8+16=24 with 4 bias-killed):
  - qb0-main: 4 slots, k-positions 0-3, always diagonal-masked.
      class A: block0 x its own 4 tiles; class B: block1 x orig 4-7 (diag).
  - flex: 4 slots reading a 512-col kt EXTENSION (per-core blend:
      class A copy of perm 12-15 = orig 8-11; class B copy of perm 4-7 =
      orig 0-3) against q2 (per-core blend: class A block3, class B block1
      queries). Accumulated into separate yf PSUM, then merged into qb1
      (class A) or qb0 (class B) with per-core 0/1 scales.
  - qb1-main: 12 slots, k-positions {0-7, 8-11(diag)}.
      class A: block3 x orig {0-3, 4-7, 12-15diag}; flex adds orig 8-11.
      class B: block2 x orig {4-7, 0-3, 8-11diag} complete.
The kt/qt extensions and flex V tiles are built with 2 cheap DVE blend ops
each from per-core scalars (sA = 1 for class A else 0).

The S^T span PSUM is bf16 (1 bank instead of 2; S needs no accumulation) to
free banks for yf. Softmax skips max-subtraction (logits ~N(0,0.4)).

Pipeline structure as v2: per head-pack p the K^T/Q^T/V' projection units
interleave into the previous pack's attention slots; exp on ScalarE;
AV with an appended ones-column providing softmax row sums; fast-approx
reciprocal (single DVE op) + gpsimd broadcast for normalization; x arrives
pre-transposed from host; W_qk d-tile-major; small tensors DMA'd first;
bf16 output.
"""

import numpy as np
from contextlib import ExitStack

B, T, C = 4, 2048, 1024
NH, HS = 16, 64
P = 128
NT = T // P           # 16 k-tiles per batch
NCORES = 8
VPW = NH * (HS + 1)   # 1040: V' columns (per-head 64 V cols + ones col)
KTW = T + 512         # kt width incl. 4-tile flex extension
QTW = 1024 + 512      # qt width incl. q2 flex block

# permuted 128-token tile order per class (see module docstring)
TILE_ORDER = {
    0: [0, 1, 2, 3, 4, 5, 6, 7, 12, 13, 14, 15, 8, 9, 10, 11],
    1: [4, 5, 6, 7, 0, 1, 2, 3, 8, 9, 10, 11, 12, 13, 14, 15],
}
NSLOTS = 20           # 4 qb0 + 4 flex + 12 qb1, uniform across cores


def _build_program():
    import concourse.bacc as bacc
    import concourse.tile as tile
    from concourse import mybir
    from concourse.mybir import ActivationFunctionType as AFT

    f32 = mybir.dt.float32
    bf16 = mybir.dt.bfloat16
    MULT = mybir.AluOpType.mult
    ADD = mybir.AluOpType.add

    nc = bacc.Bacc("TRN2", target_bir_lowering=False, debug=False,
                   num_devices=NCORES)

    # x^T blocks: index ts*8+c -> [128 (c-chunk rows), 512 (tokens)]
    xtd = nc.dram_tensor("xt", [32, P, 512], bf16, kind="ExternalInput").ap()
    # W_qk d-tile-major: wk[dt] = [128 (c-chunk rows), 8 c-chunks x 128 dims]
    wkd = nc.dram_tensor("wk", [16, P, 1024], bf16, kind="ExternalInput").ap()
    bqk = nc.dram_tensor("bqk", [P, 16], f32, kind="ExternalInput").ap()
    wvp = nc.dram_tensor("wvp", [C, VPW], bf16, kind="ExternalInput").ap()
    bvp = nc.dram_tensor("bvp", [P, VPW], f32, kind="ExternalInput").ap()
    wpj = nc.dram_tensor("wproj", [C, C], bf16, kind="ExternalInput").ap()
    bpj = nc.dram_tensor("bproj", [P, C], f32, kind="ExternalInput").ap()
    masks = nc.dram_tensor("masks", [4, P, 1024], bf16, kind="ExternalInput").ap()
    sfx = nc.dram_tensor("sfx", [P, 2], f32, kind="ExternalInput").ap()
    outd = nc.dram_tensor("out", [1024, C], bf16, kind="ExternalOutput").ap()

    with tile.TileContext(nc) as tc:
        with ExitStack() as octx:
            yt_pool = octx.enter_context(tc.tile_pool(name="yt", bufs=8))
            yT = [yt_pool.tile([P, 1024], bf16, tag="yt", name=f"yT{i}")
                  for i in range(8)]

            with ExitStack() as ctx:
                # ---- pools ---------------------------------------------
                xT_pool = ctx.enter_context(tc.tile_pool(name="xT", bufs=32))
                vs_pool = ctx.enter_context(tc.tile_pool(name="vs", bufs=84))
                kt_pool = ctx.enter_context(tc.tile_pool(name="ktp", bufs=2))
                qt_pool = ctx.enter_context(tc.tile_pool(name="qtp", bufs=2))
                pt_pool = ctx.enter_context(tc.tile_pool(name="pt", bufs=4))
                sm_pool = ctx.enter_context(tc.tile_pool(name="sm", bufs=3))
                # PSUM banks: span 2x2 + y 2x1 + shared 2x1 = 8. The flex
                # pass runs FIRST each pack and its yf accumulators are
                # evicted to SBUF, so qb0/qb1 reuse the same two y banks.
                span_p = ctx.enter_context(tc.tile_pool(name="span", bufs=2, space="PSUM"))
                yp_p = ctx.enter_context(tc.tile_pool(name="yp", bufs=2, space="PSUM"))
                sh_p = ctx.enter_context(tc.tile_pool(name="shp", bufs=2, space="PSUM"))

                # ---- input DMAs, in pipeline-unlock order --------------
                wk_pool = ctx.enter_context(tc.tile_pool(name="wk", bufs=16))
                wk_sb = [wk_pool.tile([P, 1024], bf16, tag="wk", name=f"wk{i}")
                         for i in range(16)]
                bq_pool = ctx.enter_context(tc.tile_pool(name="bq", bufs=1))
                bqk_sb = bq_pool.tile([P, 16], f32, tag="bqk")
                sfx_sb = bq_pool.tile([P, 2], f32, tag="sfx")
                mpool = ctx.enter_context(tc.tile_pool(name="masks", bufs=4))
                masks_sb = [mpool.tile([P, 1024], bf16, tag="mask", name=f"mask{i}")
                            for i in range(4)]

                nc.sync.dma_start(wk_sb[8][:], wkd[8])
                nc.sync.dma_start(bqk_sb[:], bqk)
                nc.sync.dma_start(sfx_sb[:], sfx)

                xT = [[None] * 8 for _ in range(4)]   # [ts][c] -> [128, 512]
                for ts in range(4):
                    for c in range(8):
                        xc = xT_pool.tile([P, 512], bf16, tag="xT",
                                          name=f"xT{ts}_{c}")
                        nc.sync.dma_start(xc[:], xtd[ts * 8 + c])
                        xT[ts][c] = xc

                nc.sync.dma_start(wk_sb[0][:], wkd[0])
                wv_pool = ctx.enter_context(tc.tile_pool(name="wvp", bufs=8))
                wvp_sb = [wv_pool.tile([P, VPW], bf16, tag="wvp", name=f"wvp{i}")
                          for i in range(8)]
                for c in range(8):
                    nc.sync.dma_start(wvp_sb[c][:], wvp[c * P:(c + 1) * P, :])
                bvp_sb = bq_pool.tile([P, VPW], f32, tag="bvp")
                nc.sync.dma_start(bvp_sb[:], bvp)
                for i in range(4):
                    nc.sync.dma_start(masks_sb[i][:], masks[i])

                for dt in list(range(9, 16)) + list(range(1, 8)):
                    nc.sync.dma_start(wk_sb[dt][:], wkd[dt])

                sA128 = sfx_sb[:, 0:1]

                # ---- qkv emission units (software pipelining) ----------
                v_sb = [[None] * NT for _ in range(4)]
                ve_sb = [[None] * 4 for _ in range(4)]
                kt_tiles = {}
                qt_tiles = {}

                def unit_v(g, s):
                    def emit():
                        n0 = 260 * g
                        ts, tt = s // 4, s % 4
                        acc = sh_p.tile([P, 512], f32, tag="shp")
                        for c in range(8):
                            nc.tensor.matmul(acc[:, 0:260],
                                             xT[ts][c][:, tt * P:(tt + 1) * P],
                                             wvp_sb[c][:, n0:n0 + 260],
                                             start=(c == 0), stop=(c == 7))
                        vt = vs_pool.tile([P, 260], bf16, tag="vs",
                                          name=f"v{g}_{s}")
                        nc.vector.tensor_add(vt[:], acc[:, 0:260],
                                             bvp_sb[:, n0:n0 + 260])
                        v_sb[g][s] = vt
                    return emit

                def unit_ve(g, j):
                    # flex V tile: class A -> copy of v[12+j], class B -> v[4+j]
                    # (blends run on GpSimd: SBUF-only, keeps the DVE queue
                    # free for the PSUM drains the PE waits on)
                    def emit():
                        va, vb = v_sb[g][12 + j], v_sb[g][4 + j]
                        d = sm_pool.tile([P, 260], bf16, tag="vd")
                        nc.vector.scalar_tensor_tensor(d[:], vb[:], -1.0, va[:],
                                                       MULT, ADD)
                        ve = vs_pool.tile([P, 260], bf16, tag="vs",
                                          name=f"ve{g}_{j}")
                        nc.vector.scalar_tensor_tensor(ve[:], d[:], sA128, vb[:],
                                                       MULT, ADD)
                        ve_sb[g][j] = ve
                    return emit

                def unit_k(p, ts):
                    def emit():
                        if p not in kt_tiles:
                            kt_tiles[p] = kt_pool.tile([P, KTW], bf16, tag="kt",
                                                       name=f"kt{p}")
                        kt = kt_tiles[p]
                        acc = sh_p.tile([P, 512], f32, tag="shp")
                        for c in range(8):
                            nc.tensor.matmul(acc[:],
                                             wk_sb[8 + p][:, c * P:(c + 1) * P],
                                             xT[ts][c][:], start=(c == 0), stop=(c == 7))
                        nc.vector.tensor_scalar_add(kt[:, ts * 512:(ts + 1) * 512],
                                                    acc[:], bqk_sb[:, 8 + p:9 + p])
                    return emit

                def unit_kext(p):
                    # kt extension: class A -> copy of perm 12-15 (cols
                    # 1536:2048), class B -> copy of perm 4-7 (cols 512:1024)
                    def emit():
                        kt = kt_tiles[p]
                        ka, kb = kt[:, 1536:2048], kt[:, 512:1024]
                        d = sm_pool.tile([P, 512], bf16, tag="ktd")
                        nc.vector.scalar_tensor_tensor(d[:], kb, -1.0, ka,
                                                       MULT, ADD)
                        nc.vector.scalar_tensor_tensor(kt[:, 2048:2560], d[:],
                                                       sA128, kb, MULT, ADD)
                    return emit

                def unit_q(p, qi):
                    def emit():
                        if p not in qt_tiles:
                            qt_tiles[p] = qt_pool.tile([P, QTW], bf16, tag="qt",
                                                       name=f"qt{p}")
                        qt = qt_tiles[p]
                        ts = (0, 2)[qi]
                        acc = sh_p.tile([P, 512], f32, tag="shp")
                        for c in range(8):
                            nc.tensor.matmul(acc[:],
                                             wk_sb[p][:, c * P:(c + 1) * P],
                                             xT[ts][c][:], start=(c == 0), stop=(c == 7))
                        nc.vector.tensor_scalar_add(qt[:, qi * 512:(qi + 1) * 512],
                                                    acc[:], bqk_sb[:, p:p + 1])
                    return emit

                def unit_qext(p):
                    # q2: class A -> block3 queries (cols 512:1024),
                    #     class B -> block1 queries (cols 0:512)
                    def emit():
                        qt = qt_tiles[p]
                        qa, qb_ = qt[:, 512:1024], qt[:, 0:512]
                        d = sm_pool.tile([P, 512], bf16, tag="qtd")
                        nc.vector.scalar_tensor_tensor(d[:], qb_, -1.0, qa,
                                                       MULT, ADD)
                        nc.vector.scalar_tensor_tensor(qt[:, 1024:1536], d[:],
                                                       sA128, qb_, MULT, ADD)
                    return emit

                def qkv_units(p):
                    # K/Q + extensions first (next pack's flex pass gates on
                    # them), V' ordered so the flex-V blends' inputs (tiles
                    # 4-7, 12-15) land early; V 0-3/8-11 fill the tail.
                    units = [unit_k(p, ts) for ts in range(4)]
                    units += [unit_q(p, qi) for qi in range(2)]
                    units += [unit_kext(p), unit_qext(p)]
                    if p % 2 == 0:
                        g = p // 2
                        units += [unit_v(g, s) for s in (4, 5, 6, 7, 12, 13, 14, 15)]
                        units += [unit_ve(g, j) for j in range(4)]
                        units += [unit_v(g, s) for s in (0, 1, 2, 3, 8, 9, 10, 11)]
                    return units

                def norm_units(p, sums):
                    # per-pack normalize: fast-approx reciprocal (single
                    # custom-DVE op; denominators are ~1e2-1e3 so ~51 ULP is
                    # far below budget) + one cast, then the broadcasts AND
                    # multiplies run entirely on GpSimd -- keeps the DVE
                    # queue clear for the PSUM drains the PE waits on.
                    units = []
                    recb = sm_pool.tile([P, 512], f32, tag="recb",
                                        name=f"recb{p}")

                    def u_recip():
                        nc.vector.reciprocal_approx_fast(recb[:], sums[:])
                    units.append(u_recip)
                    for qb in range(2):
                        for hh in range(2):
                            def u_norm(qb=qb, hh=hh):
                                qsl = slice(qb * 512, qb * 512 + 512)
                                i = qb * 2 + hh
                                rcst = sm_pool.tile([1, 512], bf16, tag="rcst")
                                nc.vector.tensor_copy(rcst[:],
                                                      recb[32 * i:32 * i + 1, :])
                                bcs = sm_pool.tile([P, 512], bf16, tag="bcs")
                                nc.gpsimd.partition_broadcast(bcs[:], rcst[:],
                                                              channels=P)
                                nc.gpsimd.tensor_mul(
                                    yT[p][hh * 64:(hh + 1) * 64, qsl],
                                    yT[p][hh * 64:(hh + 1) * 64, qsl],
                                    bcs[hh * 64:(hh + 1) * 64, :])
                            units.append(u_norm)
                    return units

                # ---- main pipeline over head-packs ---------------------
                for u in qkv_units(0):      # prologue
                    u()

                pend_norm = []
                for p in range(8):
                    pend = qkv_units(p + 1) if p < 8 - 1 else []
                    pend = pend[:8] + pend_norm + pend[8:]
                    total_u, emitted, si = len(pend), 0, 0
                    kt, qt = kt_tiles[p], qt_tiles[p]
                    g, off = p // 2, (p % 2) * 130
                    sums = sm_pool.tile([P, 512], f32, tag="sums")

                    def pace():
                        nonlocal si, emitted
                        si += 1
                        want = total_u * si // NSLOTS
                        while emitted < want:
                            pend.pop(0)()
                            emitted += 1

                    def run_pass(slot_list, ya, yb):
                        # software-pipelined pass: each slot's AV pair is
                        # emitted one slot late, so the first AV of a pass
                        # lands ~2 S+exp chains after the previous pass's y
                        # drain was enqueued on the DVE (hides the drain),
                        # and steady-state AVs never wait on their own exp.
                        n = len(slot_list)
                        pend_av = None

                        def av(pt, vt, first, last):
                            nc.tensor.matmul(ya[:], vt[:, off:off + 65],
                                             pt[:, 0:512],
                                             start=first, stop=last)
                            nc.tensor.matmul(yb[:], vt[:, off + 65:off + 130],
                                             pt[:, 512:1024],
                                             start=first, stop=last)

                        for sidx, (ksl, qlo, mi, vt) in enumerate(slot_list):
                            span = span_p.tile([P, 1024], f32, tag="span")
                            nc.tensor.matmul(span[:, 0:512], kt[0:64, ksl],
                                             qt[0:64, qlo:qlo + 512],
                                             start=True, stop=True)
                            nc.tensor.matmul(span[:, 512:1024], kt[64:128, ksl],
                                             qt[64:128, qlo:qlo + 512],
                                             start=True, stop=True)
                            pt = pt_pool.tile([P, 1024], bf16, tag="pt")
                            nc.scalar.activation(pt[:], span[:], AFT.Exp,
                                                 scale=0.125)
                            if mi >= 0:
                                nc.vector.tensor_mul(pt[:], pt[:],
                                                     masks_sb[mi][:])
                            if pend_av is not None:
                                av(*pend_av)
                            pend_av = (pt, vt, sidx == 0, sidx == n - 1)
                            pace()
                        av(*pend_av)

                    # flex pass FIRST: 4 slots on the kt extension against q2,
                    # accumulated into the y banks then evicted to SBUF so
                    # qb0/qb1 can reuse the banks.
                    yf1 = yp_p.tile([HS + 1, 512], f32, tag="yp")
                    yf2 = yp_p.tile([HS + 1, 512], f32, tag="yp")
                    run_pass([(slice(2048 + fs * P, 2048 + (fs + 1) * P), 1024,
                               -1, ve_sb[g][fs]) for fs in range(4)], yf1, yf2)
                    yfs = [sm_pool.tile([65, 512], bf16, tag="yfs",
                                        name=f"yfs{p}_{h}") for h in range(2)]
                    nc.vector.tensor_copy(yfs[0][:], yf1[:])
                    nc.vector.tensor_copy(yfs[1][:], yf2[:])

                    def merge(qb, y1, y2, scol1, scol64):
                        # yT[qb] = yf*s + y; sums row = yfden*s + denom
                        qsl = slice(qb * 512, qb * 512 + 512)
                        for hh, yy in ((0, y1), (1, y2)):
                            i = qb * 2 + hh
                            nc.vector.scalar_tensor_tensor(
                                yT[p][hh * 64:(hh + 1) * 64, qsl],
                                yfs[hh][0:64, :], scol64, yy[0:64, :],
                                MULT, ADD)
                            nc.vector.scalar_tensor_tensor(
                                sums[32 * i:32 * i + 1, :],
                                yfs[hh][64:65, :], scol1, yy[64:65, :],
                                MULT, ADD)

                    # qb0-main: 4 diagonal slots
                    y1 = yp_p.tile([HS + 1, 512], f32, tag="yp")
                    y2 = yp_p.tile([HS + 1, 512], f32, tag="yp")
                    run_pass([(slice(s * P, (s + 1) * P), 0, s, v_sb[g][s])
                              for s in range(4)], y1, y2)
                    merge(0, y1, y2, sfx_sb[64:65, 1:2], sfx_sb[0:64, 1:2])
                    # qb1-main: 12 slots (diagonal tiles at positions 8-11)
                    y1 = yp_p.tile([HS + 1, 512], f32, tag="yp")
                    y2 = yp_p.tile([HS + 1, 512], f32, tag="yp")
                    run_pass([(slice(s * P, (s + 1) * P), 512,
                               s - 8 if 8 <= s < 12 else -1, v_sb[g][s])
                              for s in [0, 8, 1, 9, 2, 10, 3, 11, 4, 5, 6, 7]],
                             y1, y2)
                    merge(1, y1, y2, sfx_sb[64:65, 0:1], sfx_sb[0:64, 0:1])

                    # normalize(p) runs interleaved into the next pack's slots
                    pend_norm = norm_units(p, sums)
                for u in pend_norm:
                    u()

            # ---------------- output projection --------------------------
            with ExitStack() as ctx:
                wp_pool = ctx.enter_context(tc.tile_pool(name="wpj", bufs=8))
                wpj_sb = [wp_pool.tile([P, C], bf16, tag="wpj", name=f"wpj{i}")
                          for i in range(8)]
                for c in range(8):
                    nc.sync.dma_start(wpj_sb[c][:], wpj[c * P:(c + 1) * P, :])
                bp_pool = ctx.enter_context(tc.tile_pool(name="bpj", bufs=1))
                bpj_sb = bp_pool.tile([P, C], f32, tag="bpj")
                nc.sync.dma_start(bpj_sb[:], bpj)

                pj_p = ctx.enter_context(tc.tile_pool(name="pj", bufs=4, space="PSUM"))
                ost = ctx.enter_context(tc.tile_pool(name="ost", bufs=3))
                for tt in range(8):
                    ot = ost.tile([P, C], bf16, tag="ost")
                    for co in range(2):
                        acc = pj_p.tile([P, 512], f32, tag="pj")
                        for c in range(8):
                            nc.tensor.matmul(acc[:], yT[c][:, tt * P:(tt + 1) * P],
                                             wpj_sb[c][:, co * 512:(co + 1) * 512],
                                             start=(c == 0), stop=(c == 7))
                        nc.vector.tensor_add(ot[:, co * 512:(co + 1) * 512], acc[:],
                                             bpj_sb[:, co * 512:(co + 1) * 512])
                    nc.sync.dma_start(outd[tt * P:(tt + 1) * P, :], ot[:])

    nc.compile()
    return nc


_NC_CACHE = None


def _get_program():
    global _NC_CACHE
    if _NC_CACHE is None:
        _NC_CACHE = _build_program()
    return _NC_CACHE


def _host_inputs(x, W_attn, b_attn, W_proj, b_proj):
    """Build the 8 per-core input maps."""
    import ml_dtypes
    bf = ml_dtypes.bfloat16
    x = np.asarray(x, dtype=np.float32)
    W_attn = np.asarray(W_attn, dtype=np.float32)
    b_attn = np.asarray(b_attn, dtype=np.float32)
    W_proj = np.asarray(W_proj, dtype=np.float32)
    b_proj = np.asarray(b_proj, dtype=np.float32)

    # W_qk d-tile-major: wk[dt][p, c*128+j] = W_attn[c*128+p, dt*128+j]
    wqk_nat = W_attn[:, :2 * C]                       # [1024, 2048]
    wk = np.empty((16, P, 1024), np.float32)
    for dt in range(16):
        blk = wqk_nat[:, dt * P:(dt + 1) * P]          # [1024(c), 128(dims)]
        wk[dt] = blk.reshape(8, P, P).transpose(1, 0, 2).reshape(P, 1024)
    wk = wk.astype(bf)
    bqk = np.empty((P, 16), np.float32)
    for dt in range(16):
        bqk[:, dt] = b_attn[dt * P:(dt + 1) * P]
    # V' weights: per head 64 V columns + one zero column (ones come via bias)
    wvp = np.zeros((C, VPW), np.float32)
    bvp_row = np.zeros(VPW, np.float32)
    for h in range(NH):
        wvp[:, h * 65:h * 65 + 64] = W_attn[:, 2 * C + h * HS:2 * C + (h + 1) * HS]
        bvp_row[h * 65:h * 65 + 64] = b_attn[2 * C + h * HS:2 * C + (h + 1) * HS]
        bvp_row[h * 65 + 64] = 1.0
    wvp = wvp.astype(bf)
    bvp = np.tile(bvp_row, (P, 1))
    bpj = np.tile(b_proj, (P, 1))
    wpj = W_proj.astype(bf)

    # universal diagonal masks: mask_i[k, q] = 1 if 128*i + k <= q (dup for 2 heads)
    msk = np.zeros((4, P, 1024), np.float32)
    kk = np.arange(P)[:, None]
    qq = np.arange(512)[None, :]
    for i in range(4):
        m = (P * i + kk <= qq).astype(np.float32)
        msk[i, :, 0:512] = m
        msk[i, :, 512:1024] = m
    msk = msk.astype(bf)

    in_maps = []
    for core in range(NCORES):
        b, xh = core // 2, core % 2
        order = TILE_ORDER[xh]
        tok = np.concatenate([np.arange(t * P, (t + 1) * P) for t in order])
        xc = np.ascontiguousarray(x[b][tok]).astype(bf)     # [2048, 1024]
        # x^T blocks [ts*8+c] = [128 (c rows), 512 (tokens)]
        xt = np.ascontiguousarray(
            xc.T.reshape(8, P, 4, 512).transpose(2, 0, 1, 3).reshape(32, P, 512))
        # per-core blend/merge scalars: col0 = sA (1 iff class A), col1 = sB
        sfxa = np.zeros((P, 2), np.float32)
        sfxa[:, 0] = 1.0 if xh == 0 else 0.0
        sfxa[:, 1] = 0.0 if xh == 0 else 1.0
        in_maps.append({
            "xt": xt, "wk": wk, "bqk": bqk, "wvp": wvp, "bvp": bvp,
            "wproj": wpj, "bproj": bpj, "masks": msk, "sfx": sfxa,
        })
    return in_maps


def run(inputs, trace=False, tmpdir=None):
    from concourse.bass_utils import run_bass_kernel_spmd
    nc = _get_program()
    in_maps = _host_inputs(**inputs)
    res = run_bass_kernel_spmd(nc, in_maps, core_ids=list(range(NCORES)),
                               trace=trace, tmpdir=tmpdir)
    out = np.empty((B, T, C), np.float32)
    for core in range(NCORES):
        b, xh = core // 2, core % 2
        o = np.asarray(res.results[core]["out"], dtype=np.float32)
        blk0, blk1 = (0, 3) if xh == 0 else (1, 2)
        out[b, blk0 * 512:(blk0 + 1) * 512] = o[0:512]
        out[b, blk1 * 512:(blk1 + 1) * 512] = o[512:1024]
    return out, res


def kernel(x, W_attn, b_attn, W_proj, b_proj):
    out, _ = run(dict(x=x, W_attn=W_attn, b_attn=b_attn,
                      W_proj=W_proj, b_proj=b_proj))
    return out


# revision 27
# speedup vs baseline: 1.1295x; 1.1295x over previous
"""Causal self-attention on 8 TRN2 NeuronCores (Bass/Tile, SPMD).

Problem: B=4, T=2048, C=1024, NH=16, HS=64.
  qkv = x @ W_attn + b_attn; causal softmax attention per head; y @ W_proj + b_proj.

Sharding: core = (batch b, class xh) with b = core//2, xh = core%2.
Each core computes qkv (Q^T only for its own queries) for its whole batch,
then attention + output projection for 1024 of its batch's queries: the two
512-token blocks {0,3} (class A) or {1,2} (class B) -- paired so causal work
is balanced across cores.

SPMD uniformity: all 8 cores run the *same* instruction stream; per-class
differences are absorbed into data (per-core 128-token-tile permutation of
x, shared position-universal diagonal masks, per-core 0/1 blend scalars).

v3 flex-slot schedule (20 attention slots per head-pack, NO dead slots; v2
had 8+16=24 with 4 bias-killed):
  - qb0-main: 4 slots, k-positions 0-3, always diagonal-masked.
      class A: block0 x its own 4 tiles; class B: block1 x orig 4-7 (diag).
  - flex: 4 slots reading a 512-col kt EXTENSION (per-core blend:
      class A copy of perm 12-15 = orig 8-11; class B copy of perm 4-7 =
      orig 0-3) against q2 (per-core blend: class A block3, class B block1
      queries). Accumulated into separate yf PSUM, then merged into qb1
      (class A) or qb0 (class B) with per-core 0/1 scales.
  - qb1-main: 12 slots, k-positions {0-7, 8-11(diag)}.
      class A: block3 x orig {0-3, 4-7, 12-15diag}; flex adds orig 8-11.
      class B: block2 x orig {4-7, 0-3, 8-11diag} complete.
The kt/qt extensions and flex V tiles are built with 2 cheap DVE blend ops
each from per-core scalars (sA = 1 for class A else 0).

The S^T span PSUM is bf16 (1 bank instead of 2; S needs no accumulation) to
free banks for yf. Softmax skips max-subtraction (logits ~N(0,0.4)).

Pipeline structure as v2: per head-pack p the K^T/Q^T/V' projection units
interleave into the previous pack's attention slots; exp on ScalarE;
AV with an appended ones-column providing softmax row sums; fast-approx
reciprocal (single DVE op) + gpsimd broadcast for normalization; x arrives
pre-transposed from host; W_qk d-tile-major; small tensors DMA'd first;
bf16 output.
"""

import numpy as np
from contextlib import ExitStack

B, T, C = 4, 2048, 1024
NH, HS = 16, 64
P = 128
NT = T // P           # 16 k-tiles per batch
NCORES = 8
VPW = NH * (HS + 1)   # 1040: V' columns (per-head 64 V cols + ones col)
KTW = T + 512         # kt width incl. 4-tile flex extension
QTW = 1024 + 512      # qt width incl. q2 flex block

# permuted 128-token tile order per class (see module docstring)
TILE_ORDER = {
    0: [0, 1, 2, 3, 4, 5, 6, 7, 12, 13, 14, 15, 8, 9, 10, 11],
    1: [4, 5, 6, 7, 0, 1, 2, 3, 8, 9, 10, 11, 12, 13, 14, 15],
}
NSLOTS = 20           # 4 qb0 + 4 flex + 12 qb1, uniform across cores


def _build_program():
    import concourse.bacc as bacc
    import concourse.tile as tile
    from concourse import mybir
    from concourse.mybir import ActivationFunctionType as AFT

    f32 = mybir.dt.float32
    bf16 = mybir.dt.bfloat16
    MULT = mybir.AluOpType.mult
    ADD = mybir.AluOpType.add

    nc = bacc.Bacc("TRN2", target_bir_lowering=False, debug=False,
                   num_devices=NCORES)

    # x^T blocks: index ts*8+c -> [128 (c-chunk rows), 512 (tokens)]
    xtd = nc.dram_tensor("xt", [32, P, 512], bf16, kind="ExternalInput").ap()
    # W_qk d-tile-major: wk[dt] = [128 (c-chunk rows), 8 c-chunks x 128 dims]
    wkd = nc.dram_tensor("wk", [16, P, 1024], bf16, kind="ExternalInput").ap()
    bqk = nc.dram_tensor("bqk", [P, 16], f32, kind="ExternalInput").ap()
    wvp = nc.dram_tensor("wvp", [C, VPW], bf16, kind="ExternalInput").ap()
    bvp = nc.dram_tensor("bvp", [P, VPW], f32, kind="ExternalInput").ap()
    wpj = nc.dram_tensor("wproj", [C, C], bf16, kind="ExternalInput").ap()
    bpj = nc.dram_tensor("bproj", [P, C], f32, kind="ExternalInput").ap()
    masks = nc.dram_tensor("masks", [4, P, 1024], bf16, kind="ExternalInput").ap()
    sfx = nc.dram_tensor("sfx", [P, 2], f32, kind="ExternalInput").ap()
    outd = nc.dram_tensor("out", [1024, C], bf16, kind="ExternalOutput").ap()

    with tile.TileContext(nc) as tc:
        with ExitStack() as octx:
            yt_pool = octx.enter_context(tc.tile_pool(name="yt", bufs=8))
            yT = [yt_pool.tile([P, 1024], bf16, tag="yt", name=f"yT{i}")
                  for i in range(8)]

            with ExitStack() as ctx:
                # ---- pools ---------------------------------------------
                xT_pool = ctx.enter_context(tc.tile_pool(name="xT", bufs=32))
                vs_pool = ctx.enter_context(tc.tile_pool(name="vs", bufs=84))
                kt_pool = ctx.enter_context(tc.tile_pool(name="ktp", bufs=2))
                qt_pool = ctx.enter_context(tc.tile_pool(name="qtp", bufs=2))
                pt_pool = ctx.enter_context(tc.tile_pool(name="pt", bufs=4))
                sm_pool = ctx.enter_context(tc.tile_pool(name="sm", bufs=3))
                # PSUM banks: span 2x2 + y 2x1 + shared 2x1 = 8. The flex
                # pass runs FIRST each pack and its yf accumulators are
                # evicted to SBUF, so qb0/qb1 reuse the same two y banks.
                span_p = ctx.enter_context(tc.tile_pool(name="span", bufs=2, space="PSUM"))
                yp_p = ctx.enter_context(tc.tile_pool(name="yp", bufs=2, space="PSUM"))
                sh_p = ctx.enter_context(tc.tile_pool(name="shp", bufs=2, space="PSUM"))

                # ---- input DMAs, in pipeline-unlock order --------------
                wk_pool = ctx.enter_context(tc.tile_pool(name="wk", bufs=16))
                wk_sb = [wk_pool.tile([P, 1024], bf16, tag="wk", name=f"wk{i}")
                         for i in range(16)]
                bq_pool = ctx.enter_context(tc.tile_pool(name="bq", bufs=1))
                bqk_sb = bq_pool.tile([P, 16], f32, tag="bqk")
                sfx_sb = bq_pool.tile([P, 2], f32, tag="sfx")
                mpool = ctx.enter_context(tc.tile_pool(name="masks", bufs=4))
                masks_sb = [mpool.tile([P, 1024], bf16, tag="mask", name=f"mask{i}")
                            for i in range(4)]

                nc.sync.dma_start(wk_sb[8][:], wkd[8])
                nc.sync.dma_start(bqk_sb[:], bqk)
                nc.sync.dma_start(sfx_sb[:], sfx)

                xT = [[None] * 8 for _ in range(4)]   # [ts][c] -> [128, 512]
                for ts in range(4):
                    for c in range(8):
                        xc = xT_pool.tile([P, 512], bf16, tag="xT",
                                          name=f"xT{ts}_{c}")
                        nc.sync.dma_start(xc[:], xtd[ts * 8 + c])
                        xT[ts][c] = xc

                nc.sync.dma_start(wk_sb[0][:], wkd[0])
                wv_pool = ctx.enter_context(tc.tile_pool(name="wvp", bufs=8))
                wvp_sb = [wv_pool.tile([P, VPW], bf16, tag="wvp", name=f"wvp{i}")
                          for i in range(8)]
                for c in range(8):
                    nc.sync.dma_start(wvp_sb[c][:], wvp[c * P:(c + 1) * P, :])
                bvp_sb = bq_pool.tile([P, VPW], f32, tag="bvp")
                nc.sync.dma_start(bvp_sb[:], bvp)
                for i in range(4):
                    nc.sync.dma_start(masks_sb[i][:], masks[i])

                for dt in list(range(9, 16)) + list(range(1, 8)):
                    nc.sync.dma_start(wk_sb[dt][:], wkd[dt])

                sA128 = sfx_sb[:, 0:1]

                # ---- qkv emission units (software pipelining) ----------
                v_sb = [[None] * NT for _ in range(4)]
                ve_sb = [[None] * 4 for _ in range(4)]
                kt_tiles = {}
                qt_tiles = {}

                def unit_v(g, s):
                    def emit():
                        n0 = 260 * g
                        ts, tt = s // 4, s % 4
                        acc = sh_p.tile([P, 512], f32, tag="shp")
                        for c in range(8):
                            nc.tensor.matmul(acc[:, 0:260],
                                             xT[ts][c][:, tt * P:(tt + 1) * P],
                                             wvp_sb[c][:, n0:n0 + 260],
                                             start=(c == 0), stop=(c == 7))
                        vt = vs_pool.tile([P, 260], bf16, tag="vs",
                                          name=f"v{g}_{s}")
                        nc.vector.tensor_add(vt[:], acc[:, 0:260],
                                             bvp_sb[:, n0:n0 + 260])
                        v_sb[g][s] = vt
                    return emit

                def unit_ve(g, j):
                    # flex V tile: class A -> copy of v[12+j], class B -> v[4+j]
                    # (blends run on GpSimd: SBUF-only, keeps the DVE queue
                    # free for the PSUM drains the PE waits on)
                    def emit():
                        va, vb = v_sb[g][12 + j], v_sb[g][4 + j]
                        d = sm_pool.tile([P, 260], bf16, tag="vd")
                        nc.vector.scalar_tensor_tensor(d[:], vb[:], -1.0, va[:],
                                                       MULT, ADD)
                        ve = vs_pool.tile([P, 260], bf16, tag="vs",
                                          name=f"ve{g}_{j}")
                        nc.vector.scalar_tensor_tensor(ve[:], d[:], sA128, vb[:],
                                                       MULT, ADD)
                        ve_sb[g][j] = ve
                    return emit

                def unit_k(p, ts):
                    def emit():
                        if p not in kt_tiles:
                            kt_tiles[p] = kt_pool.tile([P, KTW], bf16, tag="kt",
                                                       name=f"kt{p}")
                        kt = kt_tiles[p]
                        acc = sh_p.tile([P, 512], f32, tag="shp")
                        for c in range(8):
                            nc.tensor.matmul(acc[:],
                                             wk_sb[8 + p][:, c * P:(c + 1) * P],
                                             xT[ts][c][:], start=(c == 0), stop=(c == 7))
                        nc.vector.tensor_scalar_add(kt[:, ts * 512:(ts + 1) * 512],
                                                    acc[:], bqk_sb[:, 8 + p:9 + p])
                    return emit

                def unit_kext(p):
                    # kt extension: class A -> copy of perm 12-15 (cols
                    # 1536:2048), class B -> copy of perm 4-7 (cols 512:1024)
                    def emit():
                        kt = kt_tiles[p]
                        ka, kb = kt[:, 1536:2048], kt[:, 512:1024]
                        d = sm_pool.tile([P, 512], bf16, tag="ktd")
                        nc.vector.scalar_tensor_tensor(d[:], kb, -1.0, ka,
                                                       MULT, ADD)
                        nc.vector.scalar_tensor_tensor(kt[:, 2048:2560], d[:],
                                                       sA128, kb, MULT, ADD)
                    return emit

                def unit_q(p, qi):
                    def emit():
                        if p not in qt_tiles:
                            qt_tiles[p] = qt_pool.tile([P, QTW], bf16, tag="qt",
                                                       name=f"qt{p}")
                        qt = qt_tiles[p]
                        ts = (0, 2)[qi]
                        acc = sh_p.tile([P, 512], f32, tag="shp")
                        for c in range(8):
                            nc.tensor.matmul(acc[:],
                                             wk_sb[p][:, c * P:(c + 1) * P],
                                             xT[ts][c][:], start=(c == 0), stop=(c == 7))
                        nc.vector.tensor_scalar_add(qt[:, qi * 512:(qi + 1) * 512],
                                                    acc[:], bqk_sb[:, p:p + 1])
                    return emit

                def unit_qext(p):
                    # q2: class A -> block3 queries (cols 512:1024),
                    #     class B -> block1 queries (cols 0:512)
                    def emit():
                        qt = qt_tiles[p]
                        qa, qb_ = qt[:, 512:1024], qt[:, 0:512]
                        d = sm_pool.tile([P, 512], bf16, tag="qtd")
                        nc.vector.scalar_tensor_tensor(d[:], qb_, -1.0, qa,
                                                       MULT, ADD)
                        nc.vector.scalar_tensor_tensor(qt[:, 1024:1536], d[:],
                                                       sA128, qb_, MULT, ADD)
                    return emit

                def qkv_units(p):
                    # K/Q + extensions first (next pack's flex pass gates on
                    # them), V' ordered so the flex-V blends' inputs (tiles
                    # 4-7, 12-15) land early; V 0-3/8-11 fill the tail.
                    units = [unit_k(p, ts) for ts in range(4)]
                    units += [unit_q(p, qi) for qi in range(2)]
                    units += [unit_kext(p), unit_qext(p)]
                    if p % 2 == 0:
                        g = p // 2
                        units += [unit_v(g, s) for s in (4, 5, 6, 7, 12, 13, 14, 15)]
                        units += [unit_ve(g, j) for j in range(4)]
                        units += [unit_v(g, s) for s in (0, 1, 2, 3, 8, 9, 10, 11)]
                    return units

                def norm_units(p, sums):
                    # per-pack normalize: fast-approx reciprocal (single
                    # custom-DVE op; denominators are ~1e2-1e3 so ~51 ULP is
                    # far below budget) + one cast, then the broadcasts AND
                    # multiplies run entirely on GpSimd -- keeps the DVE
                    # queue clear for the PSUM drains the PE waits on.
                    recb = sm_pool.tile([P, 512], f32, tag="recb",
                                        name=f"recb{p}")

                    def u_recip(lo):
                        def emit():
                            nc.vector.reciprocal_approx_fast(
                                recb[lo:lo + 64, :], sums[lo:lo + 64, :])
                        return emit

                    def u_norm(qb, hh):
                        def emit():
                            qsl = slice(qb * 512, qb * 512 + 512)
                            i = qb * 2 + hh
                            rcst = sm_pool.tile([1, 512], bf16, tag="rcst")
                            nc.vector.tensor_copy(rcst[:],
                                                  recb[32 * i:32 * i + 1, :])
                            bcs = sm_pool.tile([P, 512], bf16, tag="bcs")
                            nc.gpsimd.partition_broadcast(bcs[:], rcst[:],
                                                          channels=P)
                            nc.vector.tensor_mul(
                                yT[p][hh * 64:(hh + 1) * 64, qsl],
                                yT[p][hh * 64:(hh + 1) * 64, qsl],
                                bcs[hh * 64:(hh + 1) * 64, :])
                        return emit

                    # per-qb halves: [0:3] only need merge(0) done, [3:] need
                    # merge(1) -- lets the last pack normalize qb0 mid-pack so
                    # the output projection starts earlier.
                    return [u_recip(0), u_norm(0, 0), u_norm(0, 1),
                            u_recip(64), u_norm(1, 0), u_norm(1, 1)]

                # ---- main pipeline over head-packs ---------------------
                for u in qkv_units(0):      # prologue
                    u()

                pend_norm = []
                for p in range(8):
                    pend = qkv_units(p + 1) if p < 8 - 1 else []
                    pend = pend[:8] + pend_norm + pend[8:]
                    total_u, emitted, si = len(pend), 0, 0
                    kt, qt = kt_tiles[p], qt_tiles[p]
                    g, off = p // 2, (p % 2) * 130
                    sums = sm_pool.tile([P, 512], f32, tag="sums")

                    def pace():
                        # front-loaded: all interleaved units are emitted by
                        # slot 16, so the DVE queue is drained by the time the
                        # end-of-pack y merges (which gate the next pack's
                        # flex AVs on PSUM bank reuse) are enqueued.
                        nonlocal si, emitted
                        si += 1
                        want = min(total_u, total_u * si // (NSLOTS - 4))
                        while emitted < want:
                            pend.pop(0)()
                            emitted += 1

                    def run_pass(slot_list, ya, yb):
                        # software-pipelined pass: each slot's AV pair is
                        # emitted one slot late, so the first AV of a pass
                        # lands ~2 S+exp chains after the previous pass's y
                        # drain was enqueued on the DVE (hides the drain),
                        # and steady-state AVs never wait on their own exp.
                        n = len(slot_list)
                        pend_av = None

                        def av(pt, vt, first, last):
                            nc.tensor.matmul(ya[:], vt[:, off:off + 65],
                                             pt[:, 0:512],
                                             start=first, stop=last)
                            nc.tensor.matmul(yb[:], vt[:, off + 65:off + 130],
                                             pt[:, 512:1024],
                                             start=first, stop=last)

                        for sidx, (ksl, qlo, mi, vt) in enumerate(slot_list):
                            span = span_p.tile([P, 1024], f32, tag="span")
                            nc.tensor.matmul(span[:, 0:512], kt[0:64, ksl],
                                             qt[0:64, qlo:qlo + 512],
                                             start=True, stop=True)
                            nc.tensor.matmul(span[:, 512:1024], kt[64:128, ksl],
                                             qt[64:128, qlo:qlo + 512],
                                             start=True, stop=True)
                            pt = pt_pool.tile([P, 1024], bf16, tag="pt")
                            nc.scalar.activation(pt[:], span[:], AFT.Exp,
                                                 scale=0.125)
                            if mi >= 0:
                                nc.vector.tensor_mul(pt[:], pt[:],
                                                     masks_sb[mi][:])
                            if pend_av is not None:
                                av(*pend_av)
                            pend_av = (pt, vt, sidx == 0, sidx == n - 1)
                            pace()
                        av(*pend_av)

                    # flex pass FIRST: 4 slots on the kt extension against q2,
                    # accumulated into the y banks then evicted to SBUF so
                    # qb0/qb1 can reuse the banks.
                    yf1 = yp_p.tile([HS + 1, 512], f32, tag="yp")
                    yf2 = yp_p.tile([HS + 1, 512], f32, tag="yp")
                    run_pass([(slice(2048 + fs * P, 2048 + (fs + 1) * P), 1024,
                               -1, ve_sb[g][fs]) for fs in range(4)], yf1, yf2)
                    yfs = [sm_pool.tile([65, 512], bf16, tag="yfs",
                                        name=f"yfs{p}_{h}") for h in range(2)]
                    nc.vector.tensor_copy(yfs[0][:], yf1[:])
                    nc.vector.tensor_copy(yfs[1][:], yf2[:])

                    def merge(qb, y1, y2, scol1, scol64):
                        # yT[qb] = yf*s + y; sums row = yfden*s + denom
                        qsl = slice(qb * 512, qb * 512 + 512)
                        for hh, yy in ((0, y1), (1, y2)):
                            i = qb * 2 + hh
                            nc.vector.scalar_tensor_tensor(
                                yT[p][hh * 64:(hh + 1) * 64, qsl],
                                yfs[hh][0:64, :], scol64, yy[0:64, :],
                                MULT, ADD)
                            nc.vector.scalar_tensor_tensor(
                                sums[32 * i:32 * i + 1, :],
                                yfs[hh][64:65, :], scol1, yy[64:65, :],
                                MULT, ADD)

                    # qb0-main: 4 diagonal slots
                    y1 = yp_p.tile([HS + 1, 512], f32, tag="yp")
                    y2 = yp_p.tile([HS + 1, 512], f32, tag="yp")
                    run_pass([(slice(s * P, (s + 1) * P), 0, s, v_sb[g][s])
                              for s in range(4)], y1, y2)
                    merge(0, y1, y2, sfx_sb[64:65, 1:2], sfx_sb[0:64, 1:2])
                    if p == 7:
                        # last pack: normalize qb0 now so proj tiles 0-3
                        # aren't gated on the end-of-kernel norm chain
                        nu7 = norm_units(p, sums)
                        for u in nu7[:3]:
                            u()
                    # qb1-main: 12 slots (diagonal tiles at positions 8-11)
                    y1 = yp_p.tile([HS + 1, 512], f32, tag="yp")
                    y2 = yp_p.tile([HS + 1, 512], f32, tag="yp")
                    run_pass([(slice(s * P, (s + 1) * P), 512,
                               s - 8 if 8 <= s < 12 else -1, v_sb[g][s])
                              for s in [0, 8, 1, 9, 2, 10, 3, 11, 4, 5, 6, 7]],
                             y1, y2)
                    merge(1, y1, y2, sfx_sb[64:65, 0:1], sfx_sb[0:64, 0:1])

                    # normalize(p) runs interleaved into the next pack's slots
                    pend_norm = norm_units(p, sums) if p < 7 else nu7[3:]
                for u in pend_norm:
                    u()

            # ---------------- output projection --------------------------
            with ExitStack() as ctx:
                wp_pool = ctx.enter_context(tc.tile_pool(name="wpj", bufs=8))
                wpj_sb = [wp_pool.tile([P, C], bf16, tag="wpj", name=f"wpj{i}")
                          for i in range(8)]
                for c in range(8):
                    nc.sync.dma_start(wpj_sb[c][:], wpj[c * P:(c + 1) * P, :])
                bp_pool = ctx.enter_context(tc.tile_pool(name="bpj", bufs=1))
                bpj_sb = bp_pool.tile([P, C], f32, tag="bpj")
                nc.sync.dma_start(bpj_sb[:], bpj)

                pj_p = ctx.enter_context(tc.tile_pool(name="pj", bufs=4, space="PSUM"))
                ost = ctx.enter_context(tc.tile_pool(name="ost", bufs=3))
                for tt in range(8):
                    ot = ost.tile([P, C], bf16, tag="ost")
                    for co in range(2):
                        acc = pj_p.tile([P, 512], f32, tag="pj")
                        for c in range(8):
                            nc.tensor.matmul(acc[:], yT[c][:, tt * P:(tt + 1) * P],
                                             wpj_sb[c][:, co * 512:(co + 1) * 512],
                                             start=(c == 0), stop=(c == 7))
                        nc.vector.tensor_add(ot[:, co * 512:(co + 1) * 512], acc[:],
                                             bpj_sb[:, co * 512:(co + 1) * 512])
                    nc.sync.dma_start(outd[tt * P:(tt + 1) * P, :], ot[:])

    nc.compile()
    return nc


_NC_CACHE = None


def _get_program():
    global _NC_CACHE
    if _NC_CACHE is None:
        _NC_CACHE = _build_program()
    return _NC_CACHE


def _host_inputs(x, W_attn, b_attn, W_proj, b_proj):
    """Build the 8 per-core input maps."""
    import ml_dtypes
    bf = ml_dtypes.bfloat16
    x = np.asarray(x, dtype=np.float32)
    W_attn = np.asarray(W_attn, dtype=np.float32)
    b_attn = np.asarray(b_attn, dtype=np.float32)
    W_proj = np.asarray(W_proj, dtype=np.float32)
    b_proj = np.asarray(b_proj, dtype=np.float32)

    # W_qk d-tile-major: wk[dt][p, c*128+j] = W_attn[c*128+p, dt*128+j]
    wqk_nat = W_attn[:, :2 * C]                       # [1024, 2048]
    wk = np.empty((16, P, 1024), np.float32)
    for dt in range(16):
        blk = wqk_nat[:, dt * P:(dt + 1) * P]          # [1024(c), 128(dims)]
        wk[dt] = blk.reshape(8, P, P).transpose(1, 0, 2).reshape(P, 1024)
    wk = wk.astype(bf)
    bqk = np.empty((P, 16), np.float32)
    for dt in range(16):
        bqk[:, dt] = b_attn[dt * P:(dt + 1) * P]
    # V' weights: per head 64 V columns + one zero column (ones come via bias)
    wvp = np.zeros((C, VPW), np.float32)
    bvp_row = np.zeros(VPW, np.float32)
    for h in range(NH):
        wvp[:, h * 65:h * 65 + 64] = W_attn[:, 2 * C + h * HS:2 * C + (h + 1) * HS]
        bvp_row[h * 65:h * 65 + 64] = b_attn[2 * C + h * HS:2 * C + (h + 1) * HS]
        bvp_row[h * 65 + 64] = 1.0
    wvp = wvp.astype(bf)
    bvp = np.tile(bvp_row, (P, 1))
    bpj = np.tile(b_proj, (P, 1))
    wpj = W_proj.astype(bf)

    # universal diagonal masks: mask_i[k, q] = 1 if 128*i + k <= q (dup for 2 heads)
    msk = np.zeros((4, P, 1024), np.float32)
    kk = np.arange(P)[:, None]
    qq = np.arange(512)[None, :]
    for i in range(4):
        m = (P * i + kk <= qq).astype(np.float32)
        msk[i, :, 0:512] = m
        msk[i, :, 512:1024] = m
    msk = msk.astype(bf)

    in_maps = []
    for core in range(NCORES):
        b, xh = core // 2, core % 2
        order = TILE_ORDER[xh]
        tok = np.concatenate([np.arange(t * P, (t + 1) * P) for t in order])
        xc = np.ascontiguousarray(x[b][tok]).astype(bf)     # [2048, 1024]
        # x^T blocks [ts*8+c] = [128 (c rows), 512 (tokens)]
        xt = np.ascontiguousarray(
            xc.T.reshape(8, P, 4, 512).transpose(2, 0, 1, 3).reshape(32, P, 512))
        # per-core blend/merge scalars: col0 = sA (1 iff class A), col1 = sB
        sfxa = np.zeros((P, 2), np.float32)
        sfxa[:, 0] = 1.0 if xh == 0 else 0.0
        sfxa[:, 1] = 0.0 if xh == 0 else 1.0
        in_maps.append({
            "xt": xt, "wk": wk, "bqk": bqk, "wvp": wvp, "bvp": bvp,
            "wproj": wpj, "bproj": bpj, "masks": msk, "sfx": sfxa,
        })
    return in_maps


def run(inputs, trace=False, tmpdir=None):
    from concourse.bass_utils import run_bass_kernel_spmd
    nc = _get_program()
    in_maps = _host_inputs(**inputs)
    res = run_bass_kernel_spmd(nc, in_maps, core_ids=list(range(NCORES)),
                               trace=trace, tmpdir=tmpdir)
    out = np.empty((B, T, C), np.float32)
    for core in range(NCORES):
        b, xh = core // 2, core % 2
        o = np.asarray(res.results[core]["out"], dtype=np.float32)
        blk0, blk1 = (0, 3) if xh == 0 else (1, 2)
        out[b, blk0 * 512:(blk0 + 1) * 512] = o[0:512]
        out[b, blk1 * 512:(blk1 + 1) * 512] = o[512:1024]
    return out, res


def kernel(x, W_attn, b_attn, W_proj, b_proj):
    out, _ = run(dict(x=x, W_attn=W_attn, b_attn=b_attn,
                      W_proj=W_proj, b_proj=b_proj))
    return out
